# revision 10
# baseline (speedup 1.0000x reference)
"""HardNegativeCELoss (retrieval_knn) on 8 Trainium2 cores via Bass/Tile.

Reduction of the reference math (validated in numpy):
  d2[i,j] = ||e_i||^2 + ||c_j||^2 - 2 e_i.c_j; top-K=100 smallest d2 per row.
  PE computes m = -d2/2 = e.c - cb_sq/2 - emb_sq/2 with a K=514 augmented
  contraction ([e; 1; emb_sq] x [c; -cb_sq/2; -1/2]), so PSUM holds m directly.
  Per row the outputs only need: m_code (value at the teacher code), m_max
  (= -d2_min/2), a threshold theta* with count(m >= theta*) == 100 (found by
  regula falsi with per-row thresholds; counts via fused accumulate passes),
  and S = sum_{m >= theta*} exp(-sqrt(-2m)).
  Host finalizes:
    d_code = sqrt(-2 m_code); in_top = (m_code >= theta*)
    S_corr = S + (1-in_top) * (exp(-d_code) - exp(-sqrt(-2 theta*)))
    loss_i = d_code + log(S_corr)       [= d_code - logsumexp of candidates]
    local_acc = global_acc = mean(m_code >= m_max)   [no fp ties in randn data]
    correct_in_candidates = 1.0 exactly (reference checks membership AFTER
    replacing the last candidate with the code).

Sharding: flattened token axis (12000 = 8 x 1500) across cores, codebook
replicated; per-core partial stats gathered and reduced on host.
"""

import numpy as np

B, C, T = 8, 512, 1500
V = 4096
K = 100
NT = 1536            # padded tokens per core
NTILES = 12
Z_MANY = -1.50       # seed z-scores (d2-quantile): expected counts ~274 / ~8
Z_FEW = -2.90
N_FALSI = 4

_CACHE = {}


def _build_bass():
    import concourse.bacc as bacc
    import concourse.mybir as mybir
    from concourse.tile import TileContext

    dt = mybir.dt
    Alu = mybir.AluOpType
    Act = mybir.ActivationFunctionType
    AX = mybir.AxisListType

    nc = bacc.Bacc()
    embA = nc.dram_tensor("embA", [128, NTILES * 640], dt.float32, kind="ExternalInput")
    cbtA = nc.dram_tensor("cbtA", [514, V], dt.float32, kind="ExternalInput")
    iota = nc.dram_tensor("iota", [128, V], dt.float32, kind="ExternalInput")
    codes_f = nc.dram_tensor("codes_f", [128, NTILES], dt.float32, kind="ExternalInput")
    phiA_in = nc.dram_tensor("phiA", [128, NTILES], dt.float32, kind="ExternalInput")
    phiB_in = nc.dram_tensor("phiB", [128, NTILES], dt.float32, kind="ExternalInput")

    o_names = ("o_mcode", "o_mmax", "o_theta", "o_S", "o_cnt")
    o_dram = {nm: nc.dram_tensor(nm, [128, NTILES], dt.float32, kind="ExternalOutput")
              for nm in o_names}

    with TileContext(nc) as tc:
        with (
            tc.tile_pool(name="cbt", bufs=1) as cbt_pool,
            tc.tile_pool(name="iot", bufs=1) as iota_pool,
            tc.tile_pool(name="emb", bufs=2) as emb_pool,
            tc.tile_pool(name="psum", bufs=1, space="PSUM") as psum_pool,
            tc.tile_pool(name="m", bufs=2) as m_pool,
            tc.tile_pool(name="s", bufs=1) as s_pool,
            tc.tile_pool(name="e", bufs=1) as e_pool,
            tc.tile_pool(name="wd", bufs=1) as wd_pool,
            tc.tile_pool(name="wa", bufs=1) as wa_pool,
            tc.tile_pool(name="st", bufs=1) as st_pool,
            tc.tile_pool(name="sm", bufs=2) as sm_pool,
        ):
            cbt_sb = [cbt_pool.tile([128, V], dt.float32, tag=f"cbt{k}", name=f"cbt{k}") for k in range(4)]
            cbt_sb.append(cbt_pool.tile([2, V], dt.float32, tag="cbt4", name="cbt4"))
            for k in range(4):
                nc.sync.dma_start(cbt_sb[k][:], cbtA[k * 128:(k + 1) * 128, :])
            nc.sync.dma_start(cbt_sb[4][:], cbtA[512:514, :])
            iota_sb = iota_pool.tile([128, V], dt.float32)
            nc.sync.dma_start(iota_sb[:], iota[:])

            phiA = st_pool.tile([128, NTILES], dt.float32, tag="phiA")
            phiB = st_pool.tile([128, NTILES], dt.float32, tag="phiB")
            cA = st_pool.tile([128, NTILES], dt.float32, tag="cA")
            cB = st_pool.tile([128, NTILES], dt.float32, tag="cB")
            codes_sb = st_pool.tile([128, NTILES], dt.float32, tag="codes")
            nc.sync.dma_start(phiA[:], phiA_in[:])
            nc.sync.dma_start(phiB[:], phiB_in[:])
            nc.sync.dma_start(codes_sb[:], codes_f[:])
            outs = {nm: st_pool.tile([128, NTILES], dt.float32, tag=nm, name=nm + "_sb") for nm in o_names}

            w_dve = wd_pool.tile([128, V], dt.float32)
            w_act = wa_pool.tile([128, V], dt.float32)

            def count_act(m_sb, th_col, c_col, tmp_col):
                # acc = sum_j sign(th - m_j) = #(m<th) - #(m>=th) -> c = 2048 - acc/2
                nc.scalar.activation(w_act[:], m_sb[:], Act.Sign,
                                     bias=th_col, scale=-1.0, accum_out=tmp_col)
                nc.vector.tensor_scalar(c_col, tmp_col, -0.5, 2048.0, Alu.mult, Alu.add)

            def count_dve(m_sb, th_col, c_col):
                # out = (m >= th); accum = reduce-add(out)
                nc.vector.tensor_scalar(w_dve[:], m_sb[:], th_col, 0.0,
                                        Alu.is_ge, Alu.add, accum_out=c_col)

            for j in range(NTILES):
                et = emb_pool.tile([128, 640], dt.float32, tag="et", name="et")
                nc.sync.dma_start(et[:], embA[:, j * 640:(j + 1) * 640])

                pb = [psum_pool.tile([128, 512], dt.float32, tag=f"pb{b}", name=f"pb{b}") for b in range(8)]
                for kc in range(5):
                    lhsT = et[0:2, 512:640] if kc == 4 else et[:, kc * 128:(kc + 1) * 128]
                    for b in range(8):
                        nc.tensor.matmul(pb[b][:], lhsT, cbt_sb[kc][:, b * 512:(b + 1) * 512],
                                         start=(kc == 0), stop=(kc == 4))

                m_sb = m_pool.tile([128, V], dt.float32)
                for b in range(8):
                    nc.vector.tensor_copy(m_sb[:, b * 512:(b + 1) * 512], pb[b][:])

                s_sb = s_pool.tile([128, V], dt.float32)
                e_sb = e_pool.tile([128, V], dt.float32)
                nc.scalar.activation(s_sb[:], m_sb[:], Act.Sqrt, scale=-2.0)
                nc.scalar.activation(e_sb[:], s_sb[:], Act.Exp, scale=-1.0)

                sm = [sm_pool.tile([128, 1], dt.float32, tag=f"sm{i}", name=f"sm{i}") for i in range(8)]
                pA = sm_pool.tile([128, 1], dt.float32, tag="tA", name="tA")
                pB_ = sm_pool.tile([128, 1], dt.float32, tag="tB", name="tB")
                ca = sm_pool.tile([128, 1], dt.float32, tag="tca", name="tca")
                cb_ = sm_pool.tile([128, 1], dt.float32, tag="tcb", name="tcb")
                nc.vector.tensor_scalar(pA, phiA[:, j:j + 1], 1.0, None, Alu.mult)
                nc.vector.tensor_scalar(pB_, phiB[:, j:j + 1], 1.0, None, Alu.mult)

                count_act(m_sb, pA, ca, sm[7])
                count_dve(m_sb, pB_, cb_)

                LNK = float(np.log(K))
                for it in range(2):
                    # log-secant: w = (ln cA - ln K)/(ln cA - ln max(cB,.5))
                    nc.scalar.activation(sm[0], ca, Act.Ln)
                    nc.vector.tensor_scalar(sm[1], cb_, 0.5, None, Alu.max)
                    nc.scalar.activation(sm[1], sm[1], Act.Ln)
                    nc.vector.tensor_scalar(sm[2], sm[0], sm[1], None, Alu.subtract)
                    nc.vector.reciprocal(sm[2], sm[2])
                    nc.vector.tensor_scalar(sm[0], sm[0], LNK, None, Alu.subtract)
                    nc.vector.tensor_scalar(sm[0], sm[0], sm[2], None, Alu.mult)
                    nc.vector.tensor_scalar(sm[3], pB_, pA, None, Alu.subtract)
                    nc.vector.tensor_scalar(sm[3], sm[3], sm[0], None, Alu.mult)
                    nc.vector.tensor_scalar(sm[4], sm[3], pA, None, Alu.add)    # phi_new
                    count_act(m_sb, sm[4], sm[5], sm[7])
                    nc.vector.tensor_scalar(sm[6], sm[5], float(K), None, Alu.is_ge)
                    nc.vector.tensor_scalar(sm[0], sm[4], pA, None, Alu.subtract)
                    nc.vector.scalar_tensor_tensor(pA, sm[6], sm[0], pA, Alu.mult, Alu.add)
                    nc.vector.tensor_scalar(sm[0], sm[5], ca, None, Alu.subtract)
                    nc.vector.scalar_tensor_tensor(ca, sm[6], sm[0], ca, Alu.mult, Alu.add)
                    nc.vector.tensor_scalar(sm[6], sm[6], -1.0, 1.0, Alu.mult, Alu.add)
                    nc.vector.tensor_scalar(sm[0], sm[4], pB_, None, Alu.subtract)
                    nc.vector.scalar_tensor_tensor(pB_, sm[6], sm[0], pB_, Alu.mult, Alu.add)
                    nc.vector.tensor_scalar(sm[0], sm[5], cb_, None, Alu.subtract)
                    nc.vector.scalar_tensor_tensor(cb_, sm[6], sm[0], cb_, Alu.mult, Alu.add)

                # switch to residuals f = c - K for Illinois
                fa, fb = ca, cb_
                nc.vector.tensor_scalar(fa, ca, float(K), None, Alu.subtract)
                nc.vector.tensor_scalar(fb, cb_, float(K), None, Alu.subtract)
                for it in range(N_FALSI):
                    # phi_new = phiA + fA*(phiB-phiA)/(fA-fB)
                    nc.vector.tensor_scalar(sm[0], pB_, pA, None, Alu.subtract)
                    nc.vector.tensor_scalar(sm[1], fa, fb, None, Alu.subtract)
                    nc.vector.reciprocal(sm[2], sm[1])
                    nc.vector.tensor_scalar(sm[3], fa, sm[0], None, Alu.mult)
                    nc.vector.tensor_scalar(sm[3], sm[3], sm[2], None, Alu.mult)
                    nc.vector.tensor_scalar(sm[4], sm[3], pA, None, Alu.add)    # phi_new
                    if it % 2 == 0:
                        count_act(m_sb, sm[4], sm[5], sm[7])
                    else:
                        count_dve(m_sb, sm[4], sm[5])
                    nc.vector.tensor_scalar(sm[5], sm[5], float(K), None, Alu.subtract)  # f_new
                    nc.vector.tensor_scalar(sm[6], sm[5], 0.0, None, Alu.is_ge)          # g
                    nc.vector.tensor_scalar(sm[0], sm[4], pA, None, Alu.subtract)
                    nc.vector.scalar_tensor_tensor(pA, sm[6], sm[0], pA, Alu.mult, Alu.add)
                    nc.vector.tensor_scalar(sm[1], fa, 0.5, None, Alu.mult)              # .5 fA
                    nc.vector.tensor_scalar(sm[2], sm[5], sm[1], None, Alu.subtract)
                    nc.vector.scalar_tensor_tensor(fa, sm[6], sm[2], sm[1], Alu.mult, Alu.add)
                    nc.vector.tensor_scalar(sm[6], sm[6], -1.0, 1.0, Alu.mult, Alu.add)  # 1-g
                    nc.vector.tensor_scalar(sm[0], sm[4], pB_, None, Alu.subtract)
                    nc.vector.scalar_tensor_tensor(pB_, sm[6], sm[0], pB_, Alu.mult, Alu.add)
                    nc.vector.tensor_scalar(sm[1], fb, 0.5, None, Alu.mult)
                    nc.vector.tensor_scalar(sm[2], sm[5], sm[1], None, Alu.subtract)
                    nc.vector.scalar_tensor_tensor(fb, sm[6], sm[2], sm[1], Alu.mult, Alu.add)

                th_col = outs["o_theta"][:, j:j + 1]
                nc.vector.tensor_scalar(th_col, pA, 1.0, None, Alu.mult)
                # exact count of the final mask (same is_ge comparison as the S pass)
                nc.vector.tensor_scalar(w_dve[:], m_sb[:], th_col, 0.0, Alu.is_ge, Alu.add,
                                        accum_out=outs["o_cnt"][:, j:j + 1])
                nc.vector.scalar_tensor_tensor(w_dve[:], m_sb[:], th_col, e_sb[:],
                                               Alu.is_ge, Alu.mult,
                                               accum_out=outs["o_S"][:, j:j + 1])
                nc.vector.tensor_reduce(outs["o_mmax"][:, j:j + 1], m_sb[:], AX.X, Alu.max)
                nc.vector.scalar_tensor_tensor(w_dve[:], iota_sb[:], codes_sb[:, j:j + 1], m_sb[:],
                                               Alu.is_equal, Alu.mult,
                                               accum_out=outs["o_mcode"][:, j:j + 1])

            for nm in o_names:
                nc.sync.dma_start(o_dram[nm][:], outs[nm][:])

    if not nc.is_finalized():
        nc.finalize()
    return nc


def _prep_inputs(student_emb, teacher_codes, codebook):
    emb_all = np.ascontiguousarray(np.transpose(student_emb, (0, 2, 1))
                                   ).reshape(-1, C).astype(np.float32)   # (12000, C)
    codes_all = np.asarray(teacher_codes).reshape(-1).astype(np.int64)
    cb = np.asarray(codebook, dtype=np.float32)
    cb_sq = np.sum(cb * cb, axis=1, dtype=np.float32)

    cbtA = np.empty((514, V), np.float32)
    cbtA[:C] = cb.T
    cbtA[C] = -0.5 * cb_sq
    cbtA[C + 1] = -0.5

    iota = np.tile(np.arange(V, dtype=np.float32), (128, 1))

    cbar = cb.mean(axis=0, dtype=np.float64).astype(np.float32)
    diag_var = cb.var(axis=0, dtype=np.float64).astype(np.float32)
    mean_cb_sq = float(cb_sq.mean(dtype=np.float64))
    var_cb_sq = float(cb_sq.var(dtype=np.float64))

    in_maps = []
    for k in range(B):
        e = emb_all[k * T:(k + 1) * T]                      # (1500, C)
        codes = codes_all[k * T:(k + 1) * T]
        esq = np.sum(e * e, axis=1, dtype=np.float32)

        embA = np.zeros((128, NTILES * 640), np.float32)
        eT = np.zeros((C, NT), np.float32)
        eT[:, :T] = e.T
        esq_p = np.zeros(NT, np.float32)
        esq_p[:T] = esq
        for j in range(NTILES):
            seg = embA[:, j * 640:(j + 1) * 640]
            for kc in range(4):
                seg[:, kc * 128:(kc + 1) * 128] = eT[kc * 128:(kc + 1) * 128, j * 128:(j + 1) * 128]
            seg[0, 512:640] = 1.0
            seg[1, 512:640] = esq_p[j * 128:(j + 1) * 128]

        mu = esq + mean_cb_sq - 2.0 * (e @ cbar)
        sig = np.sqrt(4.0 * ((e * e) @ diag_var) + var_cb_sq)
        phiA = (-(mu + Z_MANY * sig) * 0.5).astype(np.float32)  # theta, count >= K side
        phiB = (-(mu + Z_FEW * sig) * 0.5).astype(np.float32)   # theta, count <  K side

        def to_pt(x, fill=0.0):
            full = np.full(NT, fill, np.float32)
            full[:x.shape[0]] = x
            return full.reshape(NTILES, 128).T.copy()           # [128, NTILES]

        in_maps.append({
            "embA": embA, "cbtA": cbtA, "iota": iota,
            "codes_f": to_pt(codes.astype(np.float32)),
            "phiA": to_pt(phiA, fill=1.0),
            "phiB": to_pt(phiB, fill=2.0),
        })
    return in_maps, emb_all, codes_all


def _finalize(results):
    loss_sum = 0.0
    hit_sum = 0.0
    for k in range(B):
        r = results[k]
        def fl(nm):
            return np.asarray(r[nm]).T.reshape(NT)[:T].astype(np.float64)
        m_code, m_max, theta, S, cnt = (fl("o_mcode"), fl("o_mmax"), fl("o_theta"),
                                        fl("o_S"), fl("o_cnt"))
        d_code = np.sqrt(np.maximum(-2.0 * m_code, 0.0))
        in_top = m_code >= theta
        ehat = np.exp(-np.sqrt(np.maximum(-2.0 * theta, 0.0)))
        S_corr = S - (cnt - K) * ehat + (~in_top) * (np.exp(-d_code) - ehat)
        loss_sum += np.sum(d_code + np.log(S_corr))
        hit_sum += np.sum(m_code >= m_max)
    n = float(B * T)
    loss = np.float32(loss_sum / n)
    acc = np.float32(hit_sum / n)
    return loss, acc, acc, np.float32(1.0)


def kernel(student_emb, teacher_codes, codebook):
    from concourse.bass_utils import run_bass_kernel_spmd

    if "nc" not in _CACHE:
        _CACHE["nc"] = _build_bass()
    nc = _CACHE["nc"]
    in_maps, _, _ = _prep_inputs(np.asarray(student_emb, dtype=np.float32),
                                 teacher_codes, codebook)
    res = run_bass_kernel_spmd(nc, in_maps, core_ids=list(range(B)))
    return _finalize(res.results)
